# revision 7
# baseline (speedup 1.0000x reference)
"""2D DWT (db4, pywt 'symmetric' mode) on 8 Trainium2 NeuronCores.

Strategy: pure data-parallel over the 512 (b, c) images (64 per core).
Per image the separable transform is two banded-matrix multiplies on the
tensor engine:

    out1 = X.T @ G      (H filter pass; the X chunk is the matmul's
                         stationary operand, so the result lands with W on
                         partitions -- no transposes anywhere)
    out2 = out1.T @ G   (W filter pass; final (H2 bands x W2 bands))

The symmetric padding is folded into the band matrices (reflected taps
summed into boundary columns), so the kernel only touches the raw 512x512
image. Bands are grouped into 61-band blocks whose 8-tap footprints fit a
128-row input window, making every matmul's output columns disjoint (no
cross-matmul accumulation). Shifted input windows are built with
SBUF->SBUF DMA (cross-partition moves). Compute in fp16 (absmax rel err
~5e-4 vs fp32 reference, measured); PSUM accumulates fp32; outputs fp32.

PSUM band layout per wave: 3D tile (parts, 2, 259): group 0 = lo bands,
group 1 = hi bands, flat columns [lo 0:259 | hi 259:518]. A single matmul
writes both groups via a 2-group AP (group step 259) staying inside PSUM
bank 0 for blocks t<4; the t=4 hi tail lands in bank 1 via its own matmul.
"""

import numpy as np
from contextlib import ExitStack

import concourse.bass as bass
import concourse.tile as tile
from concourse import bacc, mybir
from concourse.bass_utils import run_bass_kernel_spmd

# ---------------- problem constants (hardcoded per contract) ----------------
B, C, H, W = 8, 64, 512, 512
N_CORES = 8
N_IMG = (B * C) // N_CORES  # 64 images per core
L = 8
J = (H + L - 1) // 2  # 259 bands per filter per axis

DEC_LO = np.array([-0.010597401784997278, 0.032883011666982945,
                   0.030841381835986965, -0.18703481171888114,
                   -0.02798376941698385, 0.6308807679295904,
                   0.7148465705525415, 0.23037781330885523], dtype=np.float64)
DEC_HI = np.array([-0.23037781330885523, 0.7148465705525415,
                   -0.6308807679295904, -0.02798376941698385,
                   0.18703481171888114, 0.030841381835986965,
                   -0.032883011666982945, -0.010597401784997278], dtype=np.float64)

# input-axis windows (all 128 rows); window t owns bands [BAND0[t], +NB[t])
WIN0 = [0, 116, 238, 360, 384]
NB = [61, 61, 61, 61, 15]
BAND0 = [0, 61, 122, 183, 244]
CONST_OFFS = [0, 122, 244, 366, 488]  # col offset of block t in packed consts

COMPUTE_DT = mybir.dt.float16
COMPUTE_NP = np.float16

# pass-2 lhsT chunks over out1's 518 flat columns
CHUNKS = [(0, 128), (128, 128), (256, 128), (384, 128), (512, 6)]


def _band_matrices():
    """M_f in R^{J x H}: out[j] = sum_h M_f[j, h] x[h] (reflection folded)."""
    def sym(t):
        if t < 0:
            return -t - 1
        if t >= H:
            return 2 * H - 1 - t
        return t

    mats = []
    for dec in (DEC_LO, DEC_HI):
        F = dec[::-1]
        M = np.zeros((J, H), dtype=np.float64)
        for j in range(J):
            for k in range(L):
                M[j, sym(2 * j + k - 6)] += F[k]
        mats.append(M)
    return mats


def pack_consts():
    """(128 x 518) fp16; block t at cols [CONST_OFFS[t] : +2*NB[t]] =
    [lo-band taps | hi-band taps] for that window."""
    M_lo, M_hi = _band_matrices()
    packed = np.zeros((128, 518), dtype=COMPUTE_NP)
    for t in range(5):
        w0, nb, b0, off = WIN0[t], NB[t], BAND0[t], CONST_OFFS[t]
        packed[:, off:off + nb] = M_lo[b0:b0 + nb, w0:w0 + 128].T
        packed[:, off + nb:off + 2 * nb] = M_hi[b0:b0 + nb, w0:w0 + 128].T
    return packed


def _chunk_pieces(c0, w):
    """Split chunk rows into (filter, h2_start, row_offset, nrows) pieces."""
    pieces = []
    c = c0
    while c < c0 + w:
        if c < J:
            n = min(J, c0 + w) - c
            pieces.append(("lo", c, c - c0, n))
        else:
            n = (c0 + w) - c
            pieces.append(("hi", c - J, c - c0, n))
        c += n
    return pieces


def _emit_wave(nc, psum, lhsT_of, cmat, nparts, sim_compat=False):
    """7 matmuls filling psum[:nparts] (a [128, 2, 259] fp32 PSUM tile).

    lhsT_of(t): stationary operand (128 x <=128, fp16) for window t.
    sim_compat: split each 2-group matmul into two contiguous ones
    (numerically identical; CoreSim rejects multi-dim matmul out APs).
    """
    for t in range(4):
        nb = NB[t]
        rhs = cmat[:, CONST_OFFS[t]:CONST_OFFS[t] + 2 * nb]
        if sim_compat:
            nc.tensor.matmul(psum[:nparts, 0, BAND0[t]:BAND0[t] + nb],
                             lhsT_of(t), rhs[:, :nb],
                             start=(t == 0), stop=False)
            nc.tensor.matmul(psum[:nparts, 1, BAND0[t]:BAND0[t] + nb],
                             lhsT_of(t), rhs[:, nb:],
                             start=False, stop=False)
        else:
            out = psum[:nparts, :, BAND0[t]:BAND0[t] + nb]  # 2-group, bank 0
            nc.tensor.matmul(out, lhsT_of(t), rhs, start=(t == 0), stop=False)
    c4 = CONST_OFFS[4]
    # t=4: lo bands 244..258 -> flat [244:259); hi 244..252 -> [503:512);
    # hi 253..258 -> [512:518) (PSUM bank 1, own accumulation group)
    nc.tensor.matmul(psum[:nparts, 0, 244:259], lhsT_of(4),
                     cmat[:, c4:c4 + 15], start=False, stop=False)
    nc.tensor.matmul(psum[:nparts, 1, 244:253], lhsT_of(4),
                     cmat[:, c4 + 15:c4 + 24], start=False, stop=True)
    nc.tensor.matmul(psum[:nparts, 1, 253:259], lhsT_of(4),
                     cmat[:, c4 + 24:c4 + 30], start=True, stop=True)


def build_program(n_img=N_IMG, sim_compat=False):
    nc = bacc.Bacc("TRN2", target_bir_lowering=False, debug=False,
                   num_devices=N_CORES)
    f32 = mybir.dt.float32
    x_ap = nc.dram_tensor("x", [n_img, H, W], f32, kind="ExternalInput").ap()
    cm_ap = nc.dram_tensor("cmat", [128, 518], COMPUTE_DT,
                           kind="ExternalInput").ap()
    out_aps = {
        z: nc.dram_tensor(z, [n_img, J, J], f32, kind="ExternalOutput").ap()
        for z in ("z0", "z1", "z2", "z3")
    }
    # (H-filter g, W-filter f) -> output: reference returns
    # z0=(rowLo,colLo) z1=(rowLo,colHi) z2=(rowHi,colLo) z3=(rowHi,colHi)
    # with row=W axis filter, col=H axis filter; here g filters H, f filters W
    zmap = {("lo", "lo"): "z0", ("hi", "lo"): "z1",
            ("lo", "hi"): "z2", ("hi", "hi"): "z3"}

    with tile.TileContext(nc) as tc, ExitStack() as ctx:
        cpool = ctx.enter_context(tc.tile_pool(name="consts", bufs=1))
        xf32p = ctx.enter_context(tc.tile_pool(name="xf32", bufs=6))
        xhp = ctx.enter_context(tc.tile_pool(name="xh", bufs=6))
        xwinp = ctx.enter_context(tc.tile_pool(name="xwin", bufs=5))
        out1p = ctx.enter_context(tc.tile_pool(name="out1", bufs=7))
        sboutp = ctx.enter_context(tc.tile_pool(name="sbout", bufs=16))
        ps1 = ctx.enter_context(tc.tile_pool(name="ps1", bufs=2, space="PSUM"))
        ps2 = ctx.enter_context(tc.tile_pool(name="ps2", bufs=2, space="PSUM"))

        cmat = cpool.tile([128, 518], COMPUTE_DT)
        nc.sync.dma_start(out=cmat[:], in_=cm_ap[:])

        for img in range(n_img):
            # ---- load image (4 aligned row tiles), cast to fp16 ----
            xf = [xf32p.tile([128, W], f32, tag="xf", name=f"xf{hb}") for hb in range(4)]
            for hb in range(4):
                nc.sync.dma_start(out=xf[hb][:],
                                  in_=x_ap[img, 128 * hb:128 * (hb + 1), :])
            xh = [xhp.tile([128, W], COMPUTE_DT, tag="xh", name=f"xh{hb}") for hb in range(4)]
            for hb in range(4):
                nc.gpsimd.tensor_copy(out=xh[hb][:], in_=xf[hb][:])
            # shifted windows 1..3 via SBUF->SBUF DMA (cross-partition)
            xw = {0: xh[0], 4: xh[3]}
            for t in (1, 2, 3):
                wt = xwinp.tile([128, W], COMPUTE_DT, tag="xw", name=f"xw{t}")
                tb, r0 = divmod(WIN0[t], 128)
                n_hi = 128 - r0
                nc.sync.dma_start(out=wt[:n_hi], in_=xh[tb][r0:128])
                nc.sync.dma_start(out=wt[n_hi:128], in_=xh[tb + 1][:r0])
                xw[t] = wt

            # ---- pass 1: out1_s = X[:, swin].T @ G ----
            out1 = []
            for s in range(5):
                p1 = ps1.tile([128, 2, J], f32, tag="p1", name=f"p1_{s}")
                _emit_wave(nc, p1,
                           lambda t: xw[t][:, WIN0[s]:WIN0[s] + 128],
                           cmat, 128, sim_compat)
                o1 = out1p.tile([128, 2 * J], COMPUTE_DT, tag="o1", name=f"o1_{s}")
                nc.scalar.copy(
                    out=o1[:].rearrange("p (g r) -> p g r", g=2, r=J),
                    in_=p1[:])
                out1.append(o1)

            # ---- pass 2 + subband extraction + store ----
            for (c0, wdt) in CHUNKS:
                p2 = ps2.tile([128, 2, J], f32, tag="p2", name=f"p2_{c0}")
                _emit_wave(nc, p2,
                           lambda s: out1[s][:, c0:c0 + wdt], cmat, wdt,
                           sim_compat)
                for fi, f in enumerate(("lo", "hi")):
                    # full-partition copy (engine APs need aligned starts);
                    # the store DMA does the row slicing freely
                    sb = sboutp.tile([128, J], f32, tag="sb",
                                     name=f"sb_{c0}_{fi}")
                    nc.vector.tensor_copy(out=sb[:wdt], in_=p2[:wdt, fi, :])
                    for (g, h2_0, row_off, nrows) in _chunk_pieces(c0, wdt):
                        nc.sync.dma_start(
                            out=out_aps[zmap[(g, f)]][
                                img, h2_0:h2_0 + nrows, :],
                            in_=sb[row_off:row_off + nrows])
    nc.compile()
    return nc


_NC_CACHE = {}


def _get_program(n_img):
    if n_img not in _NC_CACHE:
        _NC_CACHE[n_img] = build_program(n_img)
    return _NC_CACHE[n_img]


def kernel(x):
    """x: (8, 64, 512, 512) fp32 -> (LL, (LH, HL, HH)) like the reference."""
    x = np.ascontiguousarray(np.asarray(x), dtype=np.float32)
    imgs = x.reshape(B * C, H, W)
    cmat = pack_consts()
    nc = _get_program(N_IMG)
    in_maps = [
        {"x": imgs[N_IMG * i:N_IMG * (i + 1)], "cmat": cmat}
        for i in range(N_CORES)
    ]
    res = run_bass_kernel_spmd(nc, in_maps, list(range(N_CORES)))
    outs = []
    for z in ("z0", "z1", "z2", "z3"):
        full = np.concatenate([res.results[i][z] for i in range(N_CORES)])
        outs.append(full.reshape(B, C, J, J).astype(np.float32))
    z0, z1, z2, z3 = outs
    return (z0, (z1, z2, z3))


# revision 10
# speedup vs baseline: 44.0294x; 44.0294x over previous
"""2D DWT (db4, pywt 'symmetric' mode) on 8 Trainium2 NeuronCores.

Strategy: pure data-parallel over the 512 (b, c) images (64 per core).
Per image the separable transform is two banded-matrix multiplies on the
tensor engine:

    out1 = X.T @ G      (H filter pass; the X chunk is the matmul's
                         stationary operand, so the result lands with W on
                         partitions -- no transposes anywhere)
    out2 = out1.T @ G   (W filter pass; final (H2 bands x W2 bands))

The symmetric padding is folded into the band matrices (reflected taps
summed into boundary columns), so the kernel only touches the raw 512x512
image. Bands are grouped into 61-band blocks whose 8-tap footprints fit a
128-row input window, making every matmul's output columns disjoint (no
cross-matmul accumulation). Shifted input windows are built with
SBUF->SBUF DMA (cross-partition moves). Compute in fp16 (absmax rel err
~5e-4 vs fp32 reference, measured); PSUM accumulates fp32; outputs fp32.

PSUM band layout per wave: 3D tile (parts, 2, 259): group 0 = lo bands,
group 1 = hi bands, flat columns [lo 0:259 | hi 259:518]. A single matmul
writes both groups via a 2-group AP (group step 259) staying inside PSUM
bank 0 for blocks t<4; the t=4 hi tail lands in bank 1 via its own matmul.
"""

import numpy as np
from contextlib import ExitStack

import concourse.bass as bass
import concourse.tile as tile
from concourse import bacc, mybir
from concourse.bass_utils import run_bass_kernel_spmd

# ---------------- problem constants (hardcoded per contract) ----------------
B, C, H, W = 8, 64, 512, 512
N_CORES = 8
N_IMG = (B * C) // N_CORES  # 64 images per core
L = 8
J = (H + L - 1) // 2  # 259 bands per filter per axis

DEC_LO = np.array([-0.010597401784997278, 0.032883011666982945,
                   0.030841381835986965, -0.18703481171888114,
                   -0.02798376941698385, 0.6308807679295904,
                   0.7148465705525415, 0.23037781330885523], dtype=np.float64)
DEC_HI = np.array([-0.23037781330885523, 0.7148465705525415,
                   -0.6308807679295904, -0.02798376941698385,
                   0.18703481171888114, 0.030841381835986965,
                   -0.032883011666982945, -0.010597401784997278], dtype=np.float64)

# input-axis windows (all 128 rows); window t owns bands [BAND0[t], +NB[t])
WIN0 = [0, 116, 238, 360, 384]
NB = [61, 61, 61, 61, 15]
BAND0 = [0, 61, 122, 183, 244]
CONST_OFFS = [0, 122, 244, 366, 488]  # col offset of block t in packed consts

COMPUTE_DT = mybir.dt.float16
COMPUTE_NP = np.float16

# pass-2 lhsT chunks over out1's 518 flat columns
CHUNKS = [(0, 128), (128, 128), (256, 128), (384, 128), (512, 6)]


def _band_matrices():
    """M_f in R^{J x H}: out[j] = sum_h M_f[j, h] x[h] (reflection folded)."""
    def sym(t):
        if t < 0:
            return -t - 1
        if t >= H:
            return 2 * H - 1 - t
        return t

    mats = []
    for dec in (DEC_LO, DEC_HI):
        F = dec[::-1]
        M = np.zeros((J, H), dtype=np.float64)
        for j in range(J):
            for k in range(L):
                M[j, sym(2 * j + k - 6)] += F[k]
        mats.append(M)
    return mats


def pack_consts():
    """(128 x 518) fp16; block t at cols [CONST_OFFS[t] : +2*NB[t]] =
    [lo-band taps | hi-band taps] for that window."""
    M_lo, M_hi = _band_matrices()
    packed = np.zeros((128, 518), dtype=COMPUTE_NP)
    for t in range(5):
        w0, nb, b0, off = WIN0[t], NB[t], BAND0[t], CONST_OFFS[t]
        packed[:, off:off + nb] = M_lo[b0:b0 + nb, w0:w0 + 128].T
        packed[:, off + nb:off + 2 * nb] = M_hi[b0:b0 + nb, w0:w0 + 128].T
    return packed


def _chunk_pieces(c0, w):
    """Split chunk rows into (filter, h2_start, row_offset, nrows) pieces."""
    pieces = []
    c = c0
    while c < c0 + w:
        if c < J:
            n = min(J, c0 + w) - c
            pieces.append(("lo", c, c - c0, n))
        else:
            n = (c0 + w) - c
            pieces.append(("hi", c - J, c - c0, n))
        c += n
    return pieces


def _emit_wave(nc, psum, lhsT_of, cmat, nparts, sim_compat=False):
    """7 matmuls filling psum[:nparts] (a [128, 2, 259] fp32 PSUM tile).

    lhsT_of(t): stationary operand (128 x <=128, fp16) for window t.
    sim_compat: split each 2-group matmul into two contiguous ones
    (numerically identical; CoreSim rejects multi-dim matmul out APs).
    """
    for t in range(4):
        nb = NB[t]
        rhs = cmat[:, CONST_OFFS[t]:CONST_OFFS[t] + 2 * nb]
        if sim_compat:
            nc.tensor.matmul(psum[:nparts, 0, BAND0[t]:BAND0[t] + nb],
                             lhsT_of(t), rhs[:, :nb],
                             start=(t == 0), stop=False)
            nc.tensor.matmul(psum[:nparts, 1, BAND0[t]:BAND0[t] + nb],
                             lhsT_of(t), rhs[:, nb:],
                             start=False, stop=False)
        else:
            out = psum[:nparts, :, BAND0[t]:BAND0[t] + nb]  # 2-group, bank 0
            nc.tensor.matmul(out, lhsT_of(t), rhs, start=(t == 0), stop=False)
    c4 = CONST_OFFS[4]
    # t=4: lo bands 244..258 -> flat [244:259); hi 244..252 -> [503:512);
    # hi 253..258 -> [512:518) (PSUM bank 1, own accumulation group)
    nc.tensor.matmul(psum[:nparts, 0, 244:259], lhsT_of(4),
                     cmat[:, c4:c4 + 15], start=False, stop=False)
    nc.tensor.matmul(psum[:nparts, 1, 244:253], lhsT_of(4),
                     cmat[:, c4 + 15:c4 + 24], start=False, stop=True)
    nc.tensor.matmul(psum[:nparts, 1, 253:259], lhsT_of(4),
                     cmat[:, c4 + 24:c4 + 30], start=True, stop=True)


def build_program(n_img=N_IMG, sim_compat=False, repeat=1, loop_repeat=0):
    nc = bacc.Bacc("TRN2", target_bir_lowering=False, debug=False,
                   num_devices=N_CORES)
    f32 = mybir.dt.float32
    x_ap = nc.dram_tensor("x", [n_img, H, W], f32, kind="ExternalInput").ap()
    cm_ap = nc.dram_tensor("cmat", [128, 518], COMPUTE_DT,
                           kind="ExternalInput").ap()
    out_aps = {
        z: nc.dram_tensor(z, [n_img, J, J], f32, kind="ExternalOutput").ap()
        for z in ("z0", "z1", "z2", "z3")
    }
    # (H-filter g, W-filter f) -> output: reference returns
    # z0=(rowLo,colLo) z1=(rowLo,colHi) z2=(rowHi,colLo) z3=(rowHi,colHi)
    # with row=W axis filter, col=H axis filter; here g filters H, f filters W
    zmap = {("lo", "lo"): "z0", ("hi", "lo"): "z1",
            ("lo", "hi"): "z2", ("hi", "hi"): "z3"}

    with tile.TileContext(nc) as tc, ExitStack() as ctx:
        cpool = ctx.enter_context(tc.tile_pool(name="consts", bufs=1))
        xf32p = ctx.enter_context(tc.tile_pool(name="xf32", bufs=6))
        xhp = ctx.enter_context(tc.tile_pool(name="xh", bufs=6))
        xwinp = ctx.enter_context(tc.tile_pool(name="xwin", bufs=5))
        out1p = ctx.enter_context(tc.tile_pool(name="out1", bufs=7))
        sboutp = ctx.enter_context(tc.tile_pool(name="sbout", bufs=16))
        ps1 = ctx.enter_context(tc.tile_pool(name="ps1", bufs=2, space="PSUM"))
        ps2 = ctx.enter_context(tc.tile_pool(name="ps2", bufs=2, space="PSUM"))

        cmat = cpool.tile([128, 518], COMPUTE_DT)
        nc.sync.dma_start(out=cmat[:], in_=cm_ap[:])

        import contextlib
        loop_cm = (tc.For_i(0, loop_repeat, 1) if loop_repeat
                   else contextlib.nullcontext())
        with loop_cm:
         for img_i in range(n_img * repeat):
            img = img_i % n_img
            # ---- load image (4 aligned row tiles), cast to fp16 ----
            xf = [xf32p.tile([128, W], f32, tag="xf", name=f"xf{hb}") for hb in range(4)]
            for hb in range(4):
                nc.sync.dma_start(out=xf[hb][:],
                                  in_=x_ap[img, 128 * hb:128 * (hb + 1), :])
            xh = [xhp.tile([128, W], COMPUTE_DT, tag="xh", name=f"xh{hb}") for hb in range(4)]
            for hb in range(4):
                nc.gpsimd.tensor_copy(out=xh[hb][:], in_=xf[hb][:])
            # shifted windows 1..3 via SBUF->SBUF DMA (cross-partition)
            xw = {0: xh[0], 4: xh[3]}
            for t in (1, 2, 3):
                wt = xwinp.tile([128, W], COMPUTE_DT, tag="xw", name=f"xw{t}")
                tb, r0 = divmod(WIN0[t], 128)
                n_hi = 128 - r0
                nc.sync.dma_start(out=wt[:n_hi], in_=xh[tb][r0:128])
                nc.sync.dma_start(out=wt[n_hi:128], in_=xh[tb + 1][:r0])
                xw[t] = wt

            # ---- pass 1: out1_s = X[:, swin].T @ G ----
            out1 = []
            for s in range(5):
                p1 = ps1.tile([128, 2, J], f32, tag="p1", name=f"p1_{s}")
                _emit_wave(nc, p1,
                           lambda t: xw[t][:, WIN0[s]:WIN0[s] + 128],
                           cmat, 128, sim_compat)
                o1 = out1p.tile([128, 2 * J], COMPUTE_DT, tag="o1", name=f"o1_{s}")
                nc.scalar.copy(
                    out=o1[:].rearrange("p (g r) -> p g r", g=2, r=J),
                    in_=p1[:])
                out1.append(o1)

            # ---- pass 2 + subband extraction + store ----
            for (c0, wdt) in CHUNKS:
                p2 = ps2.tile([128, 2, J], f32, tag="p2", name=f"p2_{c0}")
                _emit_wave(nc, p2,
                           lambda s: out1[s][:, c0:c0 + wdt], cmat, wdt,
                           sim_compat)
                for fi, f in enumerate(("lo", "hi")):
                    # full-partition copy (engine APs need aligned starts);
                    # the store DMA does the row slicing freely
                    sb = sboutp.tile([128, J], f32, tag="sb",
                                     name=f"sb_{c0}_{fi}")
                    nc.vector.tensor_copy(out=sb[:wdt], in_=p2[:wdt, fi, :])
                    for (g, h2_0, row_off, nrows) in _chunk_pieces(c0, wdt):
                        nc.sync.dma_start(
                            out=out_aps[zmap[(g, f)]][
                                img, h2_0:h2_0 + nrows, :],
                            in_=sb[row_off:row_off + nrows])
    nc.compile()
    return nc


_NC_CACHE = {}


def _get_program(n_img):
    if n_img not in _NC_CACHE:
        _NC_CACHE[n_img] = build_program(n_img)
    return _NC_CACHE[n_img]


def kernel(x):
    """x: (8, 64, 512, 512) fp32 -> (LL, (LH, HL, HH)) like the reference."""
    x = np.ascontiguousarray(np.asarray(x), dtype=np.float32)
    imgs = x.reshape(B * C, H, W)
    cmat = pack_consts()
    nc = _get_program(N_IMG)
    in_maps = [
        {"x": imgs[N_IMG * i:N_IMG * (i + 1)], "cmat": cmat}
        for i in range(N_CORES)
    ]
    res = run_bass_kernel_spmd(nc, in_maps, list(range(N_CORES)))
    outs = []
    for z in ("z0", "z1", "z2", "z3"):
        full = np.concatenate([res.results[i][z] for i in range(N_CORES)])
        outs.append(full.reshape(B, C, J, J).astype(np.float32))
    z0, z1, z2, z3 = outs
    return (z0, (z1, z2, z3))


# revision 20
# speedup vs baseline: 62.6419x; 1.4227x over previous
"""2D DWT (db4, pywt 'symmetric' mode) on 8 Trainium2 NeuronCores.

Strategy: pure data-parallel over the 512 (b, c) images (64 per core).
Per image the separable transform is two banded-matrix multiplies on the
tensor engine:

    out1 = X.T @ G      (H filter pass; the X chunk is the matmul's
                         stationary operand, so the result lands with W on
                         partitions -- no transposes anywhere)
    out2 = out1.T @ G   (W filter pass; final (H2 bands x W2 bands))

The symmetric padding is folded into the band matrices (reflected taps
summed into boundary columns), so the kernel only touches the raw 512x512
image. Bands are grouped into 61-band blocks whose 8-tap footprints fit a
128-row input window, making every matmul's output columns disjoint (no
cross-matmul accumulation). Compute in fp16 (absmax rel err ~5e-4 vs the
fp32 reference, measured); PSUM accumulates fp32; outputs fp32.

PSUM band layout per wave: 3D tile (parts, 2, 259): group 0 = lo bands,
group 1 = hi bands. A single matmul writes both groups via a 2-group AP
(group step 259) staying inside PSUM bank 0 for blocks t<4; the t=4 hi
tail lands in bank 1 via its own matmul.

DMA dispatch (HWDGE) has a ~625ns fixed cost per dma_start, so images are
processed in blocks of IMG_BLK=4 and every DRAM transfer is batched over
the block via uniform-stride APs. Input rows are loaded per overlapping
H-window directly as fp16 (gpsimd software-DGE casts in the DMA).
"""

import contextlib
import numpy as np
from contextlib import ExitStack

import concourse.bass as bass
import concourse.tile as tile
from concourse import bacc, mybir
from concourse.bass_utils import run_bass_kernel_spmd

# ---------------- problem constants (hardcoded per contract) ----------------
B, C, H, W = 8, 64, 512, 512
N_CORES = 8
N_IMG = (B * C) // N_CORES  # 64 images per core
L = 8
J = (H + L - 1) // 2  # 259 bands per filter per axis

DEC_LO = np.array([-0.010597401784997278, 0.032883011666982945,
                   0.030841381835986965, -0.18703481171888114,
                   -0.02798376941698385, 0.6308807679295904,
                   0.7148465705525415, 0.23037781330885523], dtype=np.float64)
DEC_HI = np.array([-0.23037781330885523, 0.7148465705525415,
                   -0.6308807679295904, -0.02798376941698385,
                   0.18703481171888114, 0.030841381835986965,
                   -0.032883011666982945, -0.010597401784997278], dtype=np.float64)

# input-axis windows; window t owns bands [BAND0[t], +NB[t]) whose taps all
# fall inside rows [WIN0[t], WIN0[t]+KROWS[t])
WIN0 = [0, 116, 238, 360, 482]
KROWS = [128, 128, 128, 128, 30]
NB = [61, 61, 61, 61, 15]
BAND0 = [0, 61, 122, 183, 244]
CONST_OFFS = [0, 122, 244, 366, 488]  # col offset of block t in packed consts

COMPUTE_DT = mybir.dt.float16
COMPUTE_NP = np.float16

IMG_BLK = 4  # images per DMA batch

# pass-2 lhsT chunks over out1's 518 flat columns
CHUNKS = [(0, 128), (128, 128), (256, 128), (384, 128), (512, 6)]


def _band_matrices():
    """M_f in R^{J x H}: out[j] = sum_h M_f[j, h] x[h] (reflection folded)."""
    def sym(t):
        if t < 0:
            return -t - 1
        if t >= H:
            return 2 * H - 1 - t
        return t

    mats = []
    for dec in (DEC_LO, DEC_HI):
        F = dec[::-1]
        M = np.zeros((J, H), dtype=np.float64)
        for j in range(J):
            for k in range(L):
                M[j, sym(2 * j + k - 6)] += F[k]
        mats.append(M)
    return mats


def pack_consts():
    """(128 x 518) fp16; block t at cols [CONST_OFFS[t] : +2*NB[t]] =
    [lo-band taps | hi-band taps] for window t (rows 0:KROWS[t])."""
    M_lo, M_hi = _band_matrices()
    packed = np.zeros((128, 518), dtype=COMPUTE_NP)
    for t in range(5):
        w0, nb, b0, off, kr = WIN0[t], NB[t], BAND0[t], CONST_OFFS[t], KROWS[t]
        packed[:kr, off:off + nb] = M_lo[b0:b0 + nb, w0:w0 + kr].T
        packed[:kr, off + nb:off + 2 * nb] = M_hi[b0:b0 + nb, w0:w0 + kr].T
    return packed


def _chunk_pieces(c0, w):
    """Split chunk rows into (filter, h2_start, row_offset, nrows) pieces."""
    pieces = []
    c = c0
    while c < c0 + w:
        if c < J:
            n = min(J, c0 + w) - c
            pieces.append(("lo", c, c - c0, n))
        else:
            n = (c0 + w) - c
            pieces.append(("hi", c - J, c - c0, n))
        c += n
    return pieces


def _emit_wave_f(nc, psum, lhsT_of, cmat, nparts, fi):
    """5 matmuls filling psum[:nparts] (a [128, 259] fp32 1-bank PSUM tile)
    with all bands of one filter (fi=0 lo, fi=1 hi).

    lhsT_of(t): stationary operand (KROWS[t] x <=128, fp16) for window t.
    """
    for t in range(5):
        nb = NB[t]
        off = CONST_OFFS[t] + fi * nb
        rhs = cmat[:KROWS[t], off:off + nb]
        nc.tensor.matmul(psum[:nparts, BAND0[t]:BAND0[t] + nb],
                         lhsT_of(t), rhs,
                         start=(t == 0), stop=(t == 4))


def build_program(n_img=N_IMG, sim_compat=False, loop_repeat=0):
    assert n_img % IMG_BLK == 0
    nc = bacc.Bacc("TRN2", target_bir_lowering=False, debug=False,
                   num_devices=N_CORES)
    f32 = mybir.dt.float32
    x_ap = nc.dram_tensor("x", [n_img, H, W], f32, kind="ExternalInput").ap()
    cm_ap = nc.dram_tensor("cmat", [128, 518], COMPUTE_DT,
                           kind="ExternalInput").ap()
    out_aps = {
        z: nc.dram_tensor(z, [n_img, J, J], f32, kind="ExternalOutput").ap()
        for z in ("z0", "z1", "z2", "z3")
    }
    # (H-filter g, W-filter f) -> output tensor; matches reference ordering
    zmap = {("lo", "lo"): "z0", ("hi", "lo"): "z1",
            ("lo", "hi"): "z2", ("hi", "hi"): "z3"}

    with tile.TileContext(nc) as tc, ExitStack() as ctx:
        cpool = ctx.enter_context(tc.tile_pool(name="consts", bufs=1))
        xwinp = ctx.enter_context(tc.tile_pool(name="xwin", bufs=4))
        out1p = ctx.enter_context(tc.tile_pool(name="out1", bufs=16))
        sboutp = ctx.enter_context(tc.tile_pool(name="sbout", bufs=10))
        ps1 = ctx.enter_context(tc.tile_pool(name="ps", bufs=8, space="PSUM"))
        ps2 = ps1

        cmat = cpool.tile([128, 518], COMPUTE_DT)
        nc.sync.dma_start(out=cmat[:], in_=cm_ap[:])

        loop_cm = (tc.For_i(0, loop_repeat, 1) if loop_repeat
                   else contextlib.nullcontext())
        with loop_cm:
         for blk in range(n_img // IMG_BLK):
            i0 = blk * IMG_BLK
            # ---- load the 5 H-windows for IMG_BLK images, cast to fp16
            # during the DMA (gpsimd software DGE) ----
            xw = []
            for t in range(5):
                kr = KROWS[t]
                wt = xwinp.tile([128, IMG_BLK, W], COMPUTE_DT, tag=f"xw{t}",
                                name=f"xw{t}")
                src = x_ap[i0:i0 + IMG_BLK, WIN0[t]:WIN0[t] + kr, :]
                nc.gpsimd.dma_start(out=wt[:kr],
                                    in_=src.rearrange("i p w -> p i w"))
                xw.append(wt)

            sbtiles = {}
            for ci in range(len(CHUNKS)):
                sbtiles[ci] = sboutp.tile(
                    [128, IMG_BLK, 2, J], f32, tag="sb", name=f"sb_{ci}")

            for li in range(IMG_BLK):
                # ---- pass 1: out1_s = X[:, swin].T @ G ----
                # drains alternate DVE/ACT so consecutive waves overlap
                out1 = []
                for s in range(5):
                    ws, ks = WIN0[s], KROWS[s]
                    o1 = out1p.tile([128, 2 * J], COMPUTE_DT, tag="o1",
                                    name=f"o1_{s}")
                    for fi in range(2):
                        pf = ps1.tile([128, J], f32, tag="ps",
                                      name=f"p1_{s}_{fi}")
                        _emit_wave_f(
                            nc, pf,
                            lambda t: xw[t][:KROWS[t], li, ws:ws + ks],
                            cmat, ks, fi)
                        dst = o1[:ks, fi * J:(fi + 1) * J]
                        if (s + fi) % 2 == 0:
                            nc.vector.tensor_copy(out=dst, in_=pf[:ks])
                        else:
                            nc.scalar.copy(out=dst, in_=pf[:ks])
                    out1.append(o1)

                # ---- pass 2 + subband extraction ----
                for ci, (c0, wdt) in enumerate(CHUNKS):
                    for fi in range(2):
                        pf = ps2.tile([128, J], f32, tag="ps",
                                      name=f"p2_{c0}_{fi}")
                        _emit_wave_f(
                            nc, pf,
                            lambda t: out1[t][:KROWS[t], c0:c0 + wdt],
                            cmat, wdt, fi)
                        dst = sbtiles[ci][:wdt, li, fi, :]
                        if (ci + fi) % 2 == 0:
                            nc.scalar.copy(out=dst, in_=pf[:wdt])
                        else:
                            nc.vector.tensor_copy(out=dst, in_=pf[:wdt])

            # ---- batched stores: one DMA per (chunk, piece, filter) ----
            dma_eng = [nc.sync, nc.scalar]
            di = 0
            for ci, (c0, wdt) in enumerate(CHUNKS):
                for fi, f in enumerate(("lo", "hi")):
                    sb = sbtiles[ci]
                    for (g, h2_0, row_off, nrows) in _chunk_pieces(c0, wdt):
                        dst = out_aps[zmap[(g, f)]][
                            i0:i0 + IMG_BLK, h2_0:h2_0 + nrows, :]
                        dma_eng[di % 2].dma_start(
                            out=dst.rearrange("i h j -> h i j"),
                            in_=sb[row_off:row_off + nrows, :, fi, :])
                        di += 1
    nc.compile()
    return nc


_NC_CACHE = {}


def _get_program(n_img):
    if n_img not in _NC_CACHE:
        _NC_CACHE[n_img] = build_program(n_img)
    return _NC_CACHE[n_img]


def kernel(x):
    """x: (8, 64, 512, 512) fp32 -> (LL, (LH, HL, HH)) like the reference."""
    x = np.ascontiguousarray(np.asarray(x), dtype=np.float32)
    imgs = x.reshape(B * C, H, W)
    cmat = pack_consts()
    nc = _get_program(N_IMG)
    in_maps = [
        {"x": imgs[N_IMG * i:N_IMG * (i + 1)], "cmat": cmat}
        for i in range(N_CORES)
    ]
    res = run_bass_kernel_spmd(nc, in_maps, list(range(N_CORES)))
    outs = []
    for z in ("z0", "z1", "z2", "z3"):
        full = np.concatenate([res.results[i][z] for i in range(N_CORES)])
        outs.append(full.reshape(B, C, J, J).astype(np.float32))
    z0, z1, z2, z3 = outs
    return (z0, (z1, z2, z3))


# revision 21
# speedup vs baseline: 71.3931x; 1.1397x over previous
"""2D DWT (db4, pywt 'symmetric' mode) on 8 Trainium2 NeuronCores.

Strategy: pure data-parallel over the 512 (b, c) images (64 per core).
Per image the separable transform is two banded-matrix multiplies on the
tensor engine:

    out1 = X.T @ G      (H filter pass; the X chunk is the matmul's
                         stationary operand, so the result lands with W on
                         partitions -- no transposes anywhere)
    out2 = out1.T @ G   (W filter pass; final (H2 bands x W2 bands))

The symmetric padding is folded into the band matrices (reflected taps
summed into boundary columns), so the kernel only touches the raw 512x512
image. Bands are grouped into 61-band blocks whose 8-tap footprints fit a
128-row input window, making every matmul's output columns disjoint (no
cross-matmul accumulation). Compute in fp16 (absmax rel err ~5e-4 vs the
fp32 reference, measured); PSUM accumulates fp32; outputs fp32.

PSUM band layout per wave: 3D tile (parts, 2, 259): group 0 = lo bands,
group 1 = hi bands. A single matmul writes both groups via a 2-group AP
(group step 259) staying inside PSUM bank 0 for blocks t<4; the t=4 hi
tail lands in bank 1 via its own matmul.

DMA dispatch (HWDGE) has a ~625ns fixed cost per dma_start, so images are
processed in blocks of IMG_BLK=4 and every DRAM transfer is batched over
the block via uniform-stride APs. Input rows are loaded per overlapping
H-window directly as fp16 (gpsimd software-DGE casts in the DMA).
"""

import contextlib
import numpy as np
from contextlib import ExitStack

import concourse.bass as bass
import concourse.tile as tile
from concourse import bacc, mybir
from concourse.bass_utils import run_bass_kernel_spmd

# ---------------- problem constants (hardcoded per contract) ----------------
B, C, H, W = 8, 64, 512, 512
N_CORES = 8
N_IMG = (B * C) // N_CORES  # 64 images per core
L = 8
J = (H + L - 1) // 2  # 259 bands per filter per axis

DEC_LO = np.array([-0.010597401784997278, 0.032883011666982945,
                   0.030841381835986965, -0.18703481171888114,
                   -0.02798376941698385, 0.6308807679295904,
                   0.7148465705525415, 0.23037781330885523], dtype=np.float64)
DEC_HI = np.array([-0.23037781330885523, 0.7148465705525415,
                   -0.6308807679295904, -0.02798376941698385,
                   0.18703481171888114, 0.030841381835986965,
                   -0.032883011666982945, -0.010597401784997278], dtype=np.float64)

# input-axis windows; window t owns bands [BAND0[t], +NB[t]) whose taps all
# fall inside rows [WIN0[t], WIN0[t]+KROWS[t])
WIN0 = [0, 116, 238, 360, 482]
KROWS = [128, 128, 128, 128, 30]
NB = [61, 61, 61, 61, 15]
BAND0 = [0, 61, 122, 183, 244]
CONST_OFFS = [0, 122, 244, 366, 488]  # col offset of block t in packed consts

COMPUTE_DT = mybir.dt.float16
COMPUTE_NP = np.float16

IMG_BLK = 4  # images per DMA batch

# pass-2 lhsT chunks over out1's 518 flat columns
CHUNKS = [(0, 128), (128, 128), (256, 128), (384, 128), (512, 6)]


def _band_matrices():
    """M_f in R^{J x H}: out[j] = sum_h M_f[j, h] x[h] (reflection folded)."""
    def sym(t):
        if t < 0:
            return -t - 1
        if t >= H:
            return 2 * H - 1 - t
        return t

    mats = []
    for dec in (DEC_LO, DEC_HI):
        F = dec[::-1]
        M = np.zeros((J, H), dtype=np.float64)
        for j in range(J):
            for k in range(L):
                M[j, sym(2 * j + k - 6)] += F[k]
        mats.append(M)
    return mats


def pack_consts():
    """(128 x 518) fp16; block t at cols [CONST_OFFS[t] : +2*NB[t]] =
    [lo-band taps | hi-band taps] for window t (rows 0:KROWS[t])."""
    M_lo, M_hi = _band_matrices()
    packed = np.zeros((128, 518), dtype=COMPUTE_NP)
    for t in range(5):
        w0, nb, b0, off, kr = WIN0[t], NB[t], BAND0[t], CONST_OFFS[t], KROWS[t]
        packed[:kr, off:off + nb] = M_lo[b0:b0 + nb, w0:w0 + kr].T
        packed[:kr, off + nb:off + 2 * nb] = M_hi[b0:b0 + nb, w0:w0 + kr].T
    return packed


def _chunk_pieces(c0, w):
    """Split chunk rows into (filter, h2_start, row_offset, nrows) pieces."""
    pieces = []
    c = c0
    while c < c0 + w:
        if c < J:
            n = min(J, c0 + w) - c
            pieces.append(("lo", c, c - c0, n))
        else:
            n = (c0 + w) - c
            pieces.append(("hi", c - J, c - c0, n))
        c += n
    return pieces


def _emit_wave2(nc, psum, lhsT_of, cmat, nparts):
    """7 matmuls filling psum[:nparts] (a [128, 2, 259] fp32 PSUM tile).
    One weight load per window: each matmul writes lo+hi bands via a
    2-group AP (group step 259, inside PSUM bank 0); the t=4 hi tail
    that would cross into bank 1 gets its own matmuls."""
    for t in range(4):
        nb = NB[t]
        rhs = cmat[:, CONST_OFFS[t]:CONST_OFFS[t] + 2 * nb]
        out = psum[:nparts, :, BAND0[t]:BAND0[t] + nb]
        nc.tensor.matmul(out, lhsT_of(t), rhs, start=(t == 0), stop=False)
    c4 = CONST_OFFS[4]
    k4 = KROWS[4]
    nc.tensor.matmul(psum[:nparts, 0, 244:259], lhsT_of(4),
                     cmat[:k4, c4:c4 + 15], start=False, stop=False)
    nc.tensor.matmul(psum[:nparts, 1, 244:253], lhsT_of(4),
                     cmat[:k4, c4 + 15:c4 + 24], start=False, stop=True)
    nc.tensor.matmul(psum[:nparts, 1, 253:259], lhsT_of(4),
                     cmat[:k4, c4 + 24:c4 + 30], start=True, stop=True)


def _emit_wave_f(nc, psum, lhsT_of, cmat, nparts, fi):
    """5 matmuls filling psum[:nparts] (a [128, 259] fp32 1-bank PSUM tile)
    with all bands of one filter (fi=0 lo, fi=1 hi).

    lhsT_of(t): stationary operand (KROWS[t] x <=128, fp16) for window t.
    """
    for t in range(5):
        nb = NB[t]
        off = CONST_OFFS[t] + fi * nb
        rhs = cmat[:KROWS[t], off:off + nb]
        nc.tensor.matmul(psum[:nparts, BAND0[t]:BAND0[t] + nb],
                         lhsT_of(t), rhs,
                         start=(t == 0), stop=(t == 4))


def build_program(n_img=N_IMG, sim_compat=False, loop_repeat=0):
    assert n_img % IMG_BLK == 0
    nc = bacc.Bacc("TRN2", target_bir_lowering=False, debug=False,
                   num_devices=N_CORES)
    f32 = mybir.dt.float32
    x_ap = nc.dram_tensor("x", [n_img, H, W], f32, kind="ExternalInput").ap()
    cm_ap = nc.dram_tensor("cmat", [128, 518], COMPUTE_DT,
                           kind="ExternalInput").ap()
    out_aps = {
        z: nc.dram_tensor(z, [n_img, J, J], f32, kind="ExternalOutput").ap()
        for z in ("z0", "z1", "z2", "z3")
    }
    # (H-filter g, W-filter f) -> output tensor; matches reference ordering
    zmap = {("lo", "lo"): "z0", ("hi", "lo"): "z1",
            ("lo", "hi"): "z2", ("hi", "hi"): "z3"}

    with tile.TileContext(nc) as tc, ExitStack() as ctx:
        cpool = ctx.enter_context(tc.tile_pool(name="consts", bufs=1))
        xwinp = ctx.enter_context(tc.tile_pool(name="xwin", bufs=4))
        out1p = ctx.enter_context(tc.tile_pool(name="out1", bufs=16))
        sboutp = ctx.enter_context(tc.tile_pool(name="sbout", bufs=10))
        ps1 = ctx.enter_context(tc.tile_pool(name="ps", bufs=4, space="PSUM"))
        ps2 = ps1

        cmat = cpool.tile([128, 518], COMPUTE_DT)
        nc.sync.dma_start(out=cmat[:], in_=cm_ap[:])

        loop_cm = (tc.For_i(0, loop_repeat, 1) if loop_repeat
                   else contextlib.nullcontext())
        with loop_cm:
         for blk in range(n_img // IMG_BLK):
            i0 = blk * IMG_BLK
            # ---- load the 5 H-windows for IMG_BLK images, cast to fp16
            # during the DMA (gpsimd software DGE) ----
            xw = []
            for t in range(5):
                kr = KROWS[t]
                wt = xwinp.tile([128, IMG_BLK, W], COMPUTE_DT, tag=f"xw{t}",
                                name=f"xw{t}")
                src = x_ap[i0:i0 + IMG_BLK, WIN0[t]:WIN0[t] + kr, :]
                nc.gpsimd.dma_start(out=wt[:kr],
                                    in_=src.rearrange("i p w -> p i w"))
                xw.append(wt)

            sbtiles = {}
            for ci in range(len(CHUNKS)):
                sbtiles[ci] = sboutp.tile(
                    [128, IMG_BLK, 2, J], f32, tag="sb", name=f"sb_{ci}")

            for li in range(IMG_BLK):
                # ---- pass 1: out1_s = X[:, swin].T @ G ----
                # drains alternate DVE/ACT so consecutive waves overlap
                out1 = []
                for s in range(5):
                    ws, ks = WIN0[s], KROWS[s]
                    o1 = out1p.tile([128, 2 * J], COMPUTE_DT, tag="o1",
                                    name=f"o1_{s}")
                    pf = ps1.tile([128, 2, J], f32, tag="ps",
                                  name=f"p1_{s}")
                    _emit_wave2(
                        nc, pf,
                        lambda t: xw[t][:KROWS[t], li, ws:ws + ks],
                        cmat, ks)
                    dst = o1[:ks].rearrange("p (g r) -> p g r", g=2, r=J)
                    if s % 2 == 0:
                        nc.vector.tensor_copy(out=dst, in_=pf[:ks])
                    else:
                        nc.scalar.copy(out=dst, in_=pf[:ks])
                    out1.append(o1)

                # ---- pass 2 + subband extraction ----
                for ci, (c0, wdt) in enumerate(CHUNKS):
                    pf = ps2.tile([128, 2, J], f32, tag="ps",
                                  name=f"p2_{c0}")
                    _emit_wave2(
                        nc, pf,
                        lambda t: out1[t][:KROWS[t], c0:c0 + wdt],
                        cmat, wdt)
                    dstv = sbtiles[ci][:wdt, li, :, :]
                    if ci % 2 == 0:
                        nc.scalar.copy(out=dstv, in_=pf[:wdt])
                    else:
                        nc.vector.tensor_copy(out=dstv, in_=pf[:wdt])

            # ---- batched stores: one DMA per (chunk, piece, filter) ----
            dma_eng = [nc.sync, nc.scalar]
            di = 0
            for ci, (c0, wdt) in enumerate(CHUNKS):
                for fi, f in enumerate(("lo", "hi")):
                    sb = sbtiles[ci]
                    for (g, h2_0, row_off, nrows) in _chunk_pieces(c0, wdt):
                        dst = out_aps[zmap[(g, f)]][
                            i0:i0 + IMG_BLK, h2_0:h2_0 + nrows, :]
                        dma_eng[di % 2].dma_start(
                            out=dst.rearrange("i h j -> h i j"),
                            in_=sb[row_off:row_off + nrows, :, fi, :])
                        di += 1
    nc.compile()
    return nc


_NC_CACHE = {}


def _get_program(n_img):
    if n_img not in _NC_CACHE:
        _NC_CACHE[n_img] = build_program(n_img)
    return _NC_CACHE[n_img]


def kernel(x):
    """x: (8, 64, 512, 512) fp32 -> (LL, (LH, HL, HH)) like the reference."""
    x = np.ascontiguousarray(np.asarray(x), dtype=np.float32)
    imgs = x.reshape(B * C, H, W)
    cmat = pack_consts()
    nc = _get_program(N_IMG)
    in_maps = [
        {"x": imgs[N_IMG * i:N_IMG * (i + 1)], "cmat": cmat}
        for i in range(N_CORES)
    ]
    res = run_bass_kernel_spmd(nc, in_maps, list(range(N_CORES)))
    outs = []
    for z in ("z0", "z1", "z2", "z3"):
        full = np.concatenate([res.results[i][z] for i in range(N_CORES)])
        outs.append(full.reshape(B, C, J, J).astype(np.float32))
    z0, z1, z2, z3 = outs
    return (z0, (z1, z2, z3))
